# revision 15
# baseline (speedup 1.0000x reference)
"""Trainium2 Bass kernel for nn_Net_23210003267823 (BiGCN rumor-detection net).

Math (per branch, edge set A, weights W1,b1,W2,b2), all sym-norm folded into
host-precomputed per-edge weights w_e = dinv[src]*dinv[dst] (deg incl self):
    U  = x @ W1                                (big GEMM; x fed as bf16)
    h1 = OH_w @ U[src] + diag(dinv^2) U + b1   (weighted one-hot matmuls)
    Q  = relu(x[root]) @ W2[64:]               (root-extend folded, 128 rows)
    z  = relu(h1) @ W2[:64] + Q[batch]
    h2 = relu(OH_w @ z[src] + diag(dinv^2) z + b2)
    out_branch = [segment_mean(h2, batch) | h1[root]]
Final: log_softmax(concat(td, bu) @ fc_W + fc_b).

Sharding: nodes row-sharded over 8 cores. Message tables (raw U, then raw z)
are AllGathered in TWO HALVES (rows 0:1280 / 1280:2560 of each core) so the
per-edge dma_gathers for the lower half start while the upper half is still
being produced/gathered. Each conv phase is two passes over the 20 dst
blocks: pass A gathers+accumulates lo-half edges into 40 persistent PSUM
accumulators, pass B adds hi-half edges + the diag(self-loop) matmul and
runs the epilogue. Both convs share identical srcs/one-hot/diag tensors.
"""
import sys, os
sys.path.insert(0, "/opt/trn_rl_repo")
import numpy as np
import ml_dtypes

BF16 = ml_dtypes.bfloat16

NC_ = 8
N, E, G = 20000, 320000, 128
IN, HID, OUT = 5000, 64, 64
RPC, PRC, NBLK = 2500, 2560, 20
HPC = PRC // 2                     # 1280 rows per half
NPADH = NC_ * HPC                  # 10240 rows per gathered half-table
INP, NK = 5120, 40

_cache = {}


def _wrap16(idx):
    n = idx.shape[-1]
    out = np.zeros(idx.shape[:-1] + (128, n // 16), np.int16)
    cols = np.arange(n // 16)
    for p in range(128):
        out[..., p, :] = idx[..., cols * 16 + (p % 16)]
    return out


def _build(key):
    TBLO, TBHI = [np.asarray(a) for a in key]   # [2, NBLK] tile counts per (br, blk)
    STB = int((TBLO.sum(0) + TBHI.sum(0)).max())  # per-blk total tiles (lo0+lo1+hi0+hi1)
    MLO = int(TBLO.sum(0).max())
    MHI = int(TBHI.sum(0).max())
    KSTOP = int(os.environ.get("KSTOP", "99"))
    import concourse.bass as bass
    import concourse.mybir as mybir
    import concourse.tile as tile
    from concourse import bacc, library_config

    dt = mybir.dt
    f32, bf16, i32, i16 = dt.float32, dt.bfloat16, dt.int32, dt.int16
    AF = mybir.ActivationFunctionType
    OP = mybir.AluOpType

    nc = bacc.Bacc("TRN2", target_bir_lowering=False, debug=False, num_devices=NC_)

    # ---------------- I/O ----------------
    xc = nc.dram_tensor("xc", [RPC, IN], bf16, kind="ExternalInput")
    w1 = nc.dram_tensor("w1", [IN, 128], bf16, kind="ExternalInput")
    w2a = nc.dram_tensor("w2a", [128, 128], bf16, kind="ExternalInput")
    w2b = nc.dram_tensor("w2b", [IN, 128], bf16, kind="ExternalInput")
    bias1 = nc.dram_tensor("bias1", [128, 128], f32, kind="ExternalInput")
    bias2 = nc.dram_tensor("bias2", [128, 128], f32, kind="ExternalInput")
    srcs = nc.dram_tensor("srcs", [NBLK, 128, STB * 8], i16, kind="ExternalInput")
    ohw = nc.dram_tensor("ohw", [NBLK, 128, STB * 128], bf16, kind="ExternalInput")
    diag = nc.dram_tensor("diag", [NBLK, 128, 2 * 128], bf16, kind="ExternalInput")
    boh = nc.dram_tensor("boh", [128, NBLK, 128], f32, kind="ExternalInput")
    bidx = nc.dram_tensor("bidx", [128, PRC // 16], i16, kind="ExternalInput")
    rloc = nc.dram_tensor("rloc", [G], i32, kind="ExternalInput")
    rxloc = nc.dram_tensor("rxloc", [G], i32, kind="ExternalInput")
    fcw = nc.dram_tensor("fcw", [2, 128, 256], f32, kind="ExternalInput")
    fcb = nc.dram_tensor("fcb", [128, 2], f32, kind="ExternalInput")
    out = nc.dram_tensor("out", [G, 2], f32, kind="ExternalOutput")
    DBG = os.environ.get("KDBG", "0") == "1"
    if DBG:
        dbgU = nc.dram_tensor("dbgU", [PRC, 128], f32, kind="ExternalOutput")
        dbgZ = nc.dram_tensor("dbgZ", [PRC, 128], f32, kind="ExternalOutput")
        dbgH = nc.dram_tensor("dbgH", [PRC, 128], f32, kind="ExternalOutput")
        dbgQ = nc.dram_tensor("dbgQ", [G, 128], f32, kind="ExternalOutput")

    # ---------------- internal DRAM ----------------
    Ulo = nc.dram_tensor("Ulo", [HPC, 128], bf16)
    Uhi = nc.dram_tensor("Uhi", [HPC, 128], bf16)
    Uflo = nc.dram_tensor("Uflo", [NPADH, 128], bf16, addr_space="Shared")
    Ufhi = nc.dram_tensor("Ufhi", [NPADH, 128], bf16, addr_space="Shared")
    Zlo_ = nc.dram_tensor("Zlo", [HPC, 128], bf16)
    Zhi_ = nc.dram_tensor("Zhi", [HPC, 128], bf16)
    Zflo = nc.dram_tensor("Zflo", [NPADH, 128], bf16, addr_space="Shared")
    Zfhi = nc.dram_tensor("Zfhi", [NPADH, 128], bf16, addr_space="Shared")
    h1loc = nc.dram_tensor("h1loc", [PRC + 1, 128], f32)
    Qtab = nc.dram_tensor("Qtab", [G + 1, 128], f32, addr_space="Shared")
    qbl = nc.dram_tensor("qbl", [G, 128], f32)
    arl = nc.dram_tensor("arl", [128, 257], f32)
    arf = nc.dram_tensor("arf", [128, 257], f32, addr_space="Shared")

    RG = [list(range(NC_))]

    with tile.TileContext(nc) as tc:
        with tc.tile_pool(name="const", bufs=1) as cp:
            nc.gpsimd.load_library(library_config.mlp)

            b1t = cp.tile([128, 128], f32)
            nc.sync.dma_start(out=b1t[:], in_=bias1[:])
            b2t = cp.tile([128, 128], f32)
            nc.sync.dma_start(out=b2t[:], in_=bias2[:])
            w2at = cp.tile([128, 128], bf16)
            nc.scalar.dma_start(out=w2at[:], in_=w2a[:])
            bidxt = cp.tile([128, PRC // 16], i16)
            nc.sync.dma_start(out=bidxt[:], in_=bidx[:])
            rloct = cp.tile([128, 1], i32)
            nc.sync.dma_start(out=rloct[:], in_=rloc[:, None])
            rxloct = cp.tile([128, 1], i32)
            nc.sync.dma_start(out=rxloct[:], in_=rxloc[:, None])
            fcw0 = cp.tile([128, 256], f32)
            nc.sync.dma_start(out=fcw0[:], in_=fcw[0])
            fcw1 = cp.tile([128, 256], f32)
            nc.sync.dma_start(out=fcw1[:], in_=fcw[1])
            fcbt = cp.tile([128, 2], f32)
            nc.sync.dma_start(out=fcbt[:], in_=fcb[:])
            boht = cp.tile([128, NBLK, 128], f32)
            nc.scalar.dma_start(out=boht[:], in_=boh[:])
            Uloc = cp.tile([128, NBLK, 128], bf16)
            Zloc = cp.tile([128, NBLK, 128], bf16)
            qall = cp.tile([128, NBLK, 128], f32)

            zrow = cp.tile([1, 128], f32)
            nc.vector.memset(zrow[:], 0.0)
            nc.sync.dma_start(out=h1loc[PRC:PRC + 1, :], in_=zrow[:])
            nc.sync.dma_start(out=Qtab[G:G + 1, :], in_=zrow[:])

            # ---------------- phase R: root rows -> Q (partial) ----------------
            if KSTOP >= 1:
             with tc.tile_pool(name="pr", bufs=2) as pr, \
                 tc.tile_pool(name="prp", bufs=1, space="PSUM") as prp:
                Rt_ = pr.tile([128, INP], bf16, tag="rbig")
                nc.vector.memset(Rt_[:], 0.0)
                nc.gpsimd.indirect_dma_start(
                    out=Rt_[:, 0:IN], out_offset=None, in_=xc[:],
                    in_offset=bass.IndirectOffsetOnAxis(ap=rxloct[:, :1], axis=0),
                    bounds_check=RPC - 1, oob_is_err=False)
                Rr = pr.tile([128, INP], bf16, tag="rbig2")
                nc.scalar.activation(Rr[:], Rt_[:], AF.Relu)
                w2ball = pr.tile([128, NK * 128], bf16, tag="w2ball")
                nc.vector.memset(w2ball[:, 39 * 128:], 0.0)
                nc.sync.dma_start(out=w2ball[:, 0:39 * 128].rearrange("p (k f) -> p k f", f=128),
                                  in_=w2b[0:4992, :].rearrange("(k p) f -> p k f", p=128))
                nc.sync.dma_start(out=w2ball[0:8, 39 * 128:40 * 128], in_=w2b[4992:IN, :])
                pq = prp.tile([128, 128], f32)
                rtall = pr.tile([128, NK, 128], bf16, tag="rtall")
                nc.sync.dma_start(out=rtall[:], in_=Rr[:], transpose=True)
                for k in range(NK):
                    nc.tensor.matmul(out=pq[:], lhsT=rtall[:, k, :], rhs=w2ball[:, k * 128:(k + 1) * 128],
                                     start=(k == 0), stop=(k == NK - 1))
                qsb = pr.tile([128, 128], f32, tag="qsb")
                nc.vector.tensor_copy(qsb[:], pq[:])
                nc.sync.dma_start(out=qbl[:], in_=qsb[:])
            if KSTOP >= 1:
             nc.gpsimd.collective_compute("AllReduce", OP.add, replica_groups=RG,
                                          ins=[qbl[:]], outs=[Qtab[0:G, :]])
             # Q[batch] for all local rows; runs on gpsimd while G computes
             nc.gpsimd.dma_gather(qall[:], Qtab[:], bidxt[:], PRC, PRC, 128, single_packet=False)

            # ---------------- phase G: U = x @ W1 (raw, bf16) ----------------
            if KSTOP >= 2:
             with tc.tile_pool(name="pw", bufs=1) as pw, \
                 tc.tile_pool(name="px", bufs=5) as px, \
                 tc.tile_pool(name="pxt", bufs=2) as pxt, \
                 tc.tile_pool(name="pub", bufs=3) as pub, \
                 tc.tile_pool(name="pup", bufs=2, space="PSUM") as pup:
                w1all = pw.tile([128, NK * 128], bf16)
                nc.vector.memset(w1all[:, 39 * 128:], 0.0)
                nc.scalar.dma_start(out=w1all[:, 0:39 * 128].rearrange("p (k f) -> p k f", f=128),
                                    in_=w1[0:4992, :].rearrange("(k p) f -> p k f", p=128))
                nc.scalar.dma_start(out=w1all[0:8, 39 * 128:40 * 128], in_=w1[4992:IN, :])

                for rc in range(5):
                    xbs = []
                    for j in range(4):
                        bi = rc * 4 + j
                        row0 = bi * 128
                        nr = min(128, RPC - row0)
                        xb = px.tile([128, INP], bf16, tag="xb")
                        if nr < 128:
                            nc.vector.memset(xb[:], 0.0)
                        else:
                            nc.vector.memset(xb[:, IN:INP], 0.0)
                        nc.scalar.dma_start(out=xb[0:nr, 0:IN], in_=xc[row0:row0 + nr, :])
                        xbs.append(xb)
                    pu = pup.tile([128, 512], f32)
                    xtc = pxt.tile([128, NK, 4, 128], bf16, tag="xtc")
                    for j in range(4):
                        nc.sync.dma_start(out=xtc[:, :, j, :], in_=xbs[j][:], transpose=True)
                    for k in range(NK):
                        nc.tensor.matmul(out=pu[:], lhsT=w1all[:, k * 128:(k + 1) * 128], rhs=xtc[:, k, :, :],
                                         start=(k == 0), stop=(k == NK - 1))
                    ut = pub.tile([128, 512], bf16, tag="ut")
                    nc.vector.tensor_copy(ut[:], pu[:])
                    ubt = pub.tile([128, 4, 128], bf16, tag="ubt")
                    nc.sync.dma_start(out=ubt[:], in_=ut[:], transpose=True)
                    for j in range(4):
                        bi = rc * 4 + j
                        nc.vector.tensor_copy(Uloc[:, bi, :], ubt[:, j, :])
                        if bi < 10:
                            nc.scalar.dma_start(out=Ulo[bi * 128:(bi + 1) * 128, :], in_=ubt[:, j, :])
                        else:
                            nc.scalar.dma_start(out=Uhi[(bi - 10) * 128:(bi - 9) * 128, :], in_=ubt[:, j, :])
                    if rc == 2 and KSTOP >= 3:
                        nc.gpsimd.collective_compute("AllGather", OP.bypass, replica_groups=RG,
                                                     ins=[Ulo[:]], outs=[Uflo[:]])



            # ---------------- conv phases ----------------
            def conv_phase(tlo, thi, loc, bias_t, is_c2, mid_cc=None):
                """Two-pass conv over 20 dst blocks."""
                with tc.tile_pool(name="pa", bufs=4) as pa, \
                     tc.tile_pool(name="pv", bufs=4) as pv, \
                     tc.tile_pool(name="po", bufs=4) as po, \
                     tc.tile_pool(name="pm", bufs=3) as pm, \
                     tc.tile_pool(name="php", bufs=3, space="PSUM") as php, \
                     tc.tile_pool(name="pz", bufs=2, space="PSUM") as pz:
                    accA = pm.tile([128, NBLK * 2, 64], f32)
                    def acc(blk, br):
                        return accA[:, 2 * blk + br, :]
                    # PASS A: lo-half gathers (both branches merged per blk)
                    for blk in range(NBLK):
                        lo0, lo1 = int(TBLO[0][blk]), int(TBLO[1][blk])
                        nlo = lo0 + lo1
                        st = pa.tile([128, MLO * 8], i16, tag="stA")
                        nc.sync.dma_start(out=st[:, 0:nlo * 8], in_=srcs[blk][:, 0:nlo * 8])
                        oh = po.tile([128, MLO, 128], bf16, tag="ohA")
                        nc.scalar.dma_start(out=oh[:, 0:nlo, :],
                                            in_=ohw[blk][:, 0:nlo * 128].rearrange("e (t d) -> e t d", d=128))
                        V = pv.tile([128, MLO, 128], bf16, tag="vA")
                        nc.gpsimd.dma_gather(V[:, 0:nlo, :], tlo[:], st[:, 0:nlo * 8],
                                             nlo * 128, nlo * 128, 128, single_packet=False)
                        for br in range(2):
                            pha = php.tile([128, 64], f32, tag="pha")
                            t0 = 0 if br == 0 else lo0
                            tn = lo0 if br == 0 else lo1
                            for t in range(tn):
                                nc.tensor.matmul(out=pha[:], lhsT=oh[:, t0 + t, :],
                                                 rhs=V[:, t0 + t, br * 64:(br + 1) * 64],
                                                 start=(t == 0), stop=(t == tn - 1))
                            # park partial (+bias) in SBUF f32
                            nc.vector.tensor_tensor(out=acc(blk, br), in0=pha[:],
                                                    in1=bias_t[:, br * 64:(br + 1) * 64], op=OP.add)
                        if blk == 15 and mid_cc is not None:
                            mid_cc()
                    # PASS B: hi-half + diag + epilogue
                    if is_c2:
                        pseg = pz.tile([128, 129], f32, tag="pseg")
                    for blk in range(NBLK):
                        lo0, lo1 = int(TBLO[0][blk]), int(TBLO[1][blk])
                        hi0, hi1 = int(TBHI[0][blk]), int(TBHI[1][blk])
                        nlo, nhi = lo0 + lo1, hi0 + hi1
                        st = pa.tile([128, MHI * 8], i16, tag="stB")
                        nc.sync.dma_start(out=st[:, 0:nhi * 8], in_=srcs[blk][:, nlo * 8:(nlo + nhi) * 8])
                        oh = po.tile([128, MHI, 128], bf16, tag="ohB")
                        nc.scalar.dma_start(out=oh[:, 0:nhi, :],
                                            in_=ohw[blk][:, nlo * 128:(nlo + nhi) * 128].rearrange("e (t d) -> e t d", d=128))
                        V = pv.tile([128, MHI, 128], bf16, tag="vB")
                        nc.gpsimd.dma_gather(V[:, 0:nhi, :], thi[:], st[:, 0:nhi * 8],
                                             nhi * 128, nhi * 128, 128, single_packet=False)
                        dg = po.tile([128, 2, 128], bf16, tag="dgB")
                        nc.sync.dma_start(out=dg[:].rearrange("e b d -> e (b d)"), in_=diag[blk])
                        phbs = []
                        for br in range(2):
                            phb = php.tile([128, 64], f32, tag="phb")
                            phbs.append(phb)
                            t0 = 0 if br == 0 else hi0
                            tn = hi0 if br == 0 else hi1
                            for t in range(tn):
                                nc.tensor.matmul(out=phb[:], lhsT=oh[:, t0 + t, :],
                                                 rhs=V[:, t0 + t, br * 64:(br + 1) * 64],
                                                 start=(t == 0), stop=False)
                            nc.tensor.matmul(out=phb[:], lhsT=dg[:, br, :],
                                             rhs=loc[:, blk, br * 64:(br + 1) * 64],
                                             start=False, stop=True)
                        if not is_c2:
                            h1f = pm.tile([128, 128], f32, tag="h1f")
                            for br in range(2):
                                nc.vector.tensor_tensor(out=h1f[:, br * 64:(br + 1) * 64], in0=acc(blk, br),
                                                        in1=phbs[br][:], op=OP.add)
                            nc.scalar.dma_start(out=h1loc[blk * 128:(blk + 1) * 128, :], in_=h1f[:])
                            hr = pm.tile([128, 128], bf16, tag="hr")
                            nc.scalar.activation(hr[:], h1f[:], AF.Relu)
                            hrT = pm.tile([128, 128], bf16, tag="hrT")
                            nc.sync.dma_start(out=hrT[:], in_=hr[:], transpose=True)
                            pz_ = pz.tile([128, 128], f32, tag="pzz")
                            nc.tensor.matmul(out=pz_[:], lhsT=hrT[:], rhs=w2at[:], start=True, stop=True)
                            zf = pm.tile([128, 128], bf16, tag="zf")
                            nc.vector.tensor_tensor(out=zf[:], in0=pz_[:], in1=qall[:, blk, :], op=OP.add)
                            nc.vector.tensor_copy(Zloc[:, blk, :], zf[:])
                            if blk < 10:
                                nc.scalar.dma_start(out=Zlo_[blk * 128:(blk + 1) * 128, :], in_=zf[:])
                                if blk == 9 and KSTOP >= 5:
                                    nc.gpsimd.collective_compute("AllGather", OP.bypass, replica_groups=RG,
                                                                 ins=[Zlo_[:]], outs=[Zflo[:]])
                            else:
                                nc.scalar.dma_start(out=Zhi_[(blk - 10) * 128:(blk - 9) * 128, :], in_=zf[:])
                        else:
                            pay = pm.tile([128, 129], f32, tag="pay")
                            nc.vector.memset(pay[:, 128:129], 1.0)
                            for br in range(2):
                                hs2 = pm.tile([128, 64], f32, tag="hs2")
                                nc.vector.tensor_tensor(out=hs2[:], in0=acc(blk, br),
                                                        in1=phbs[br][:], op=OP.add)
                                nc.scalar.activation(pay[:, br * 64:(br + 1) * 64], hs2[:], AF.Relu)
                            nc.tensor.matmul(out=pseg[:], lhsT=boht[:, blk, :], rhs=pay[:],
                                             start=(blk == 0), stop=(blk == NBLK - 1))
                    if is_c2:
                        rg = pm.tile([128, 128], f32, tag="rg")
                        nc.gpsimd.indirect_dma_start(
                            out=rg[:], out_offset=None, in_=h1loc[:],
                            in_offset=bass.IndirectOffsetOnAxis(ap=rloct[:, :1], axis=0))
                        part = pm.tile([128, 257], f32, tag="part")
                        nc.vector.tensor_copy(part[:, 0:129], pseg[:])
                        nc.vector.tensor_copy(part[:, 129:257], rg[:])
                        nc.sync.dma_start(out=arl[:], in_=part[:])

            def _ag_uhi():
                if KSTOP >= 3:
                    nc.gpsimd.collective_compute("AllGather", OP.bypass, replica_groups=RG,
                                                 ins=[Uhi[:]], outs=[Ufhi[:]])

            def _ag_zhi():
                if KSTOP >= 5:
                    nc.gpsimd.collective_compute("AllGather", OP.bypass, replica_groups=RG,
                                                 ins=[Zhi_[:]], outs=[Zfhi[:]])

            if KSTOP >= 4:
                conv_phase(Uflo, Ufhi, Uloc, b1t, is_c2=False, mid_cc=_ag_uhi)
            if KSTOP >= 6:
                conv_phase(Zflo, Zfhi, Zloc, b2t, is_c2=True, mid_cc=_ag_zhi)

            if DBG and KSTOP >= 4:
             with tc.tile_pool(name="pdbg", bufs=2) as pd:
                for b in range(NBLK):
                    src_t = (Ulo if b < 10 else Uhi)
                    zrc_t = (Zlo_ if b < 10 else Zhi_)
                    ro = (b % 10) * 128
                    t1 = pd.tile([128, 128], f32, tag="t1")
                    nc.gpsimd.dma_start(out=t1[:], in_=src_t[ro:ro + 128, :])
                    nc.sync.dma_start(out=dbgU[b * 128:(b + 1) * 128, :], in_=t1[:])
                    t2 = pd.tile([128, 128], f32, tag="t2")
                    nc.gpsimd.dma_start(out=t2[:], in_=zrc_t[ro:ro + 128, :])
                    nc.sync.dma_start(out=dbgZ[b * 128:(b + 1) * 128, :], in_=t2[:])
                    t3 = pd.tile([128, 128], f32, tag="t3")
                    nc.sync.dma_start(out=t3[:], in_=h1loc[b * 128:(b + 1) * 128, :])
                    nc.sync.dma_start(out=dbgH[b * 128:(b + 1) * 128, :], in_=t3[:])
                tq = pd.tile([128, 128], f32, tag="tq")
                nc.sync.dma_start(out=tq[:], in_=Qtab[0:G, :])
                nc.sync.dma_start(out=dbgQ[:], in_=tq[:])

            if KSTOP >= 7:
             nc.gpsimd.collective_compute("AllReduce", OP.add, replica_groups=RG,
                                          ins=[arl[:]], outs=[arf[:]])

            # ---------------- final ----------------
            if KSTOP >= 7:
             with tc.tile_pool(name="pf", bufs=1) as pf:
                Rt = pf.tile([128, 257], f32)
                nc.sync.dma_start(out=Rt[:], in_=arf[:])
                cnt = Rt[:, 128:129]
                c1 = pf.tile([128, 1], f32)
                nc.vector.tensor_scalar_max(out=c1[:], in0=cnt, scalar1=1.0)
                rec = pf.tile([128, 1], f32)
                nc.vector.reciprocal(rec[:], c1[:])
                ind = pf.tile([128, 1], f32)
                nc.vector.tensor_scalar_min(out=ind[:], in0=cnt, scalar1=1.0)
                hfc = pf.tile([128, 256], f32)
                nc.vector.tensor_scalar(out=hfc[:, 0:64], in0=Rt[:, 0:64], scalar1=rec[:, :1], scalar2=None, op0=OP.mult)
                nc.vector.tensor_scalar(out=hfc[:, 64:128], in0=Rt[:, 129:193], scalar1=ind[:, :1], scalar2=None, op0=OP.mult)
                nc.vector.tensor_scalar(out=hfc[:, 128:192], in0=Rt[:, 64:128], scalar1=rec[:, :1], scalar2=None, op0=OP.mult)
                nc.vector.tensor_scalar(out=hfc[:, 192:256], in0=Rt[:, 193:257], scalar1=ind[:, :1], scalar2=None, op0=OP.mult)
                lg = pf.tile([128, 2], f32)
                for j, fw in enumerate((fcw0, fcw1)):
                    tmp = pf.tile([128, 256], f32, tag=f"tmp{j}")
                    nc.vector.tensor_tensor(out=tmp[:], in0=hfc[:], in1=fw[:], op=OP.mult)
                    nc.vector.reduce_sum(lg[:, j:j + 1], tmp[:], axis=mybir.AxisListType.X)
                nc.vector.tensor_tensor(out=lg[:], in0=lg[:], in1=fcbt[:], op=OP.add)
                mx = pf.tile([128, 1], f32)
                nc.vector.reduce_max(mx[:], lg[:], axis=mybir.AxisListType.X)
                d_ = pf.tile([128, 2], f32)
                nc.vector.tensor_scalar(out=d_[:], in0=lg[:], scalar1=mx[:, :1], scalar2=None, op0=OP.subtract)
                e_ = pf.tile([128, 2], f32)
                nc.scalar.activation(e_[:], d_[:], AF.Exp)
                s_ = pf.tile([128, 1], f32)
                nc.vector.reduce_sum(s_[:], e_[:], axis=mybir.AxisListType.X)
                ls = pf.tile([128, 1], f32)
                nc.scalar.activation(ls[:], s_[:], AF.Ln)
                ov = pf.tile([128, 2], f32)
                nc.vector.tensor_scalar(out=ov[:], in0=d_[:], scalar1=ls[:, :1], scalar2=None, op0=OP.subtract)
                nc.sync.dma_start(out=out[:], in_=ov[:])

    nc.compile()
    return nc


def _prep(x, edge_index, bu_edge_index, batch, root_index,
          W1_td, b1_td, W2_td, b2_td, W1_bu, b1_bu, W2_bu, b2_bu, fc_W, fc_b):
    """Host-side: index metadata, normalization weights, parameter reshaping."""
    x = np.asarray(x, np.float32)
    batch = np.asarray(batch).astype(np.int64)
    root_index = np.asarray(root_index).astype(np.int64)
    edges = [np.asarray(edge_index).astype(np.int64), np.asarray(bu_edge_index).astype(np.int64)]

    dinvs = []
    for ei in edges:
        d = (np.bincount(ei[1], minlength=N) + 1).astype(np.float64)
        dinvs.append((1.0 / np.sqrt(d)).astype(np.float32))

    # per (core, br, blk, half): edge sublists
    sub = [[[None] * NBLK for _ in range(2)] for _ in range(NC_)]
    for br, ei in enumerate(edges):
        src, dst = ei[0], ei[1]
        w = dinvs[br][src] * dinvs[br][dst]
        c = dst // RPC
        loc = dst - c * RPC
        blk = loc // 128
        rel = loc - blk * 128
        srcloc = src - (src // RPC) * RPC
        half = (srcloc >= HPC).astype(np.int64)
        ps = (src // RPC) * HPC + (srcloc - half * HPC)   # row in the half-table
        key = (c * NBLK + blk) * 2 + half
        order = np.argsort(key, kind="stable")
        ks = key[order]
        bounds = np.searchsorted(ks, np.arange(NC_ * NBLK * 2 + 1))
        for c_ in range(NC_):
            for b_ in range(NBLK):
                pair = []
                for h_ in range(2):
                    k_ = (c_ * NBLK + b_) * 2 + h_
                    sl = order[bounds[k_]:bounds[k_ + 1]]
                    pair.append((ps[sl], rel[sl], w[sl]))
                sub[c_][br][b_] = pair

    TBLO = np.zeros((2, NBLK), np.int64)
    TBHI = np.zeros((2, NBLK), np.int64)
    for br in range(2):
        for b in range(NBLK):
            TBLO[br][b] = max(1, max((len(sub[c][br][b][0][0]) + 127) // 128 for c in range(NC_)))
            TBHI[br][b] = max(1, max((len(sub[c][br][b][1][0]) + 127) // 128 for c in range(NC_)))
    STB = int((TBLO.sum(0) + TBHI.sum(0)).max())

    srcs_flat = np.zeros((NC_, NBLK, STB * 128), np.int64)
    ohw = np.zeros((NC_, NBLK, 128, STB * 128), BF16)
    for c in range(NC_):
        for b in range(NBLK):
            off = 0
            for h in range(2):
                for br in range(2):
                    s, r, w = sub[c][br][b][h]
                    tb = int((TBLO if h == 0 else TBHI)[br][b])
                    n = len(s)
                    srcs_flat[c, b, off:off + n] = s
                    lane, til = np.arange(n) % 128, np.arange(n) // 128
                    ohw[c, b, lane, (off // 128 + til) * 128 + r] = w.astype(BF16)
                    off += tb * 128
    srcs16 = _wrap16(srcs_flat.reshape(NC_ * NBLK, STB * 128)).reshape(NC_, NBLK, 128, STB * 8)

    diag = np.zeros((NC_, NBLK, 128, 2 * 128), BF16)
    lanes = np.arange(128)
    for br in range(2):
        d2 = (dinvs[br] * dinvs[br]).reshape(NC_, RPC)
        for c in range(NC_):
            for b in range(NBLK):
                rows = np.arange(b * 128, min((b + 1) * 128, RPC))
                nr = len(rows)
                diag[c, b, lanes[:nr], br * 128 + lanes[:nr]] = d2[c, rows].astype(BF16)

    bohs = np.zeros((NC_, 128, NBLK, 128), np.float32)
    bpc = batch.reshape(NC_, RPC)
    for c in range(NC_):
        for b in range(NBLK):
            rows = np.arange(b * 128, min((b + 1) * 128, RPC))
            bohs[c, lanes[:len(rows)], b, bpc[c, rows]] = 1.0

    bidx_flat = np.full((NC_, PRC), G, np.int64)
    bidx_flat[:, :RPC] = batch.reshape(NC_, RPC)
    bidx16 = _wrap16(bidx_flat)

    rc = root_index // RPC
    rl = root_index - rc * RPC
    rloc = np.full((NC_, G), PRC, np.int32)
    rxloc = np.full((NC_, G), 1 << 20, np.int32)
    for g in range(G):
        rloc[rc[g], g] = rl[g]
        rxloc[rc[g], g] = rl[g]

    xb = x.astype(BF16)
    w1 = np.hstack([np.asarray(W1_td, np.float32), np.asarray(W1_bu, np.float32)]).astype(BF16)
    w2a = np.zeros((128, 128), np.float32)
    w2a[0:64, 0:64] = np.asarray(W2_td, np.float32)[:HID]
    w2a[64:128, 64:128] = np.asarray(W2_bu, np.float32)[:HID]
    w2a = w2a.astype(BF16)
    w2b = np.hstack([np.asarray(W2_td, np.float32)[HID:], np.asarray(W2_bu, np.float32)[HID:]]).astype(BF16)
    bias1 = np.broadcast_to(np.concatenate([np.asarray(b1_td, np.float32), np.asarray(b1_bu, np.float32)]), (128, 128)).copy()
    bias2 = np.broadcast_to(np.concatenate([np.asarray(b2_td, np.float32), np.asarray(b2_bu, np.float32)]), (128, 128)).copy()
    fcw = np.stack([np.broadcast_to(np.asarray(fc_W, np.float32)[:, j], (128, 256)) for j in range(2)])
    fcb = np.broadcast_to(np.asarray(fc_b, np.float32), (128, 2)).copy()

    in_maps = []
    for c in range(NC_):
        in_maps.append(dict(
            xc=np.ascontiguousarray(xb[c * RPC:(c + 1) * RPC]),
            w1=w1, w2a=w2a, w2b=w2b, bias1=bias1, bias2=bias2,
            srcs=np.ascontiguousarray(srcs16[c]),
            ohw=np.ascontiguousarray(ohw[c]),
            diag=np.ascontiguousarray(diag[c]),
            boh=np.ascontiguousarray(bohs[c]),
            bidx=np.ascontiguousarray(bidx16[c]),
            rloc=np.ascontiguousarray(rloc[c]), rxloc=np.ascontiguousarray(rxloc[c]),
            fcw=np.ascontiguousarray(fcw), fcb=fcb,
        ))
    key = (tuple(map(tuple, TBLO.tolist())), tuple(map(tuple, TBHI.tolist())))
    return key, in_maps


def kernel(**inputs):
    from concourse.bass_utils import run_bass_kernel_spmd
    key, in_maps = _prep(**inputs)
    if key not in _cache:
        _cache[key] = _build(key)
    nc = _cache[key]
    res = run_bass_kernel_spmd(nc, in_maps, list(range(NC_)))
    return res.results[0]["out"]


if __name__ == "__main__":
    import reference
    inputs = {k: np.asarray(v) for k, v in reference.setup_inputs().items()}
    got = kernel(**inputs)
    print(got[:4])
